# revision 4
# baseline (speedup 1.0000x reference)
"""GroupPretrainHead on 8 NeuronCores (Trainium2, Bass/Tile).

Expert-parallel sharding: core g owns group g's decoder (W[g], b[g]) and
processes exactly the samples routed to group g. The host does the routing
permutation (gather/scatter of rows = the MoE dispatch/combine step); the
device does all FLOPs: out.T = W[g] @ h.T + b[g] as a K-accumulated matmul.

v2: h/W shipped as bf16 (halves HBM traffic, 4x PE rate vs fp32; PSUM
accumulation stays fp32). hT DMAs are chunked KCH k-tiles per transfer and
alternated across the two HWDGE rings (sync + scalar) to saturate HBM.
Output columns are packed two n-chunks deep into 128 partitions (PE writes
the odd chunk at PSUM partition offset 64) so the single SWDGE writeback
runs at full DMA width with one semaphore wait.

Device-side layout per core (C = max group count, rounded up to 128):
  hT    [16, 128, C]   bf16 -- gathered hidden rows, transposed, k-tile major
  wT    [128, 16*64]   bf16 -- W[g] transposed to [d-partition, (ktile j)]
  bias2 [128, 1]       f32  -- b[g] stacked twice
  outT  [128, C2]      f32  -- preds.T, n-chunks 2i/2i+1 at partitions 0/64
"""

import numpy as np
import ml_dtypes

N_GROUPS = 8
D_MODEL = 2048
MAX_GS = 64
PART = 128
KT = D_MODEL // PART  # 16
KCH = 2  # k-tiles per DMA chunk
NCH = 512  # matmul n-chunk (one PSUM bank of f32)

TRACE = False
LAST_EXEC_NS = None
LAST_RESULTS = None

_nc_cache = {}


def _chunks(C):
    """n-chunk (offset, size) pairs and the packed outT column count."""
    offs = [(o, min(NCH, C - o)) for o in range(0, C, NCH)]
    c2 = 0
    for i in range(0, len(offs), 2):
        c2 += offs[i][1]
    return offs, c2


def _make_tile_context_cls():
    import concourse.mybir as mybir
    from concourse.tile import TileContext
    from concourse.vector_clock import ScopedClock

    class SplitDrainTileContext(TileContext):
        """This container's walrus encodes at most ONE semaphore wait per
        instruction; Tile's kernel-tail drain aggregates every outstanding
        sem onto a single InstDrain, which fails codegen. Split it into a
        chain of one-wait drains."""

        def _drain_and_barrier(self, tick_clock, wait_clock):
            drain_inst = self.nc.sync.drain()
            wait_clock.add_sem_waits(
                drain_inst.ins, ScopedClock({None: tick_clock.global_clock})
            )
            si = drain_inst.ins.sync_info
            waits = list(si.on_wait) if si else []
            if len(waits) > 1:
                si.on_wait = waits[:1]
                drain_inst.ins.sync_info = si
                for w in waits[1:]:
                    d2 = self.nc.sync.drain()
                    d2.ins.sync_info = mybir.SyncInfo(on_wait=[w], on_update=[])
            self.nc.all_engine_barrier()
            popped = self.nc._tile_sem_poison_stack.pop()
            assert popped is self._sem_poison
            self.nc.clear_and_free_semaphores(list(self.sems.allocated().values()))
            self.nc.all_engine_barrier()

    return SplitDrainTileContext


def _build_nc(C):
    import concourse.bass as bass
    import concourse.mybir as mybir

    TileContext = _make_tile_context_cls()

    f32 = mybir.dt.float32
    bf16 = mybir.dt.bfloat16
    nc = bass.Bass()

    offs, C2 = _chunks(C)

    hT = nc.declare_dram_parameter("hT", [KT, PART, C], bf16, isOutput=False)
    wT = nc.declare_dram_parameter("wT", [PART, KT * MAX_GS], bf16, isOutput=False)
    bias2 = nc.declare_dram_parameter("bias2", [PART, 1], f32, isOutput=False)
    outT = nc.declare_dram_parameter("outT", [PART, C2], f32, isOutput=True)

    # The two HWDGE rings: SP (sync) and ACT (scalar).
    def ring(i):
        return nc.sync if i % 2 == 0 else nc.scalar

    with TileContext(nc) as tc:
        with (
            tc.tile_pool(name="const", bufs=1) as constp,
            tc.tile_pool(name="h", bufs=KT // KCH) as hp,
            tc.tile_pool(name="psum", bufs=1, space=bass.MemorySpace.PSUM) as pp,
            tc.tile_pool(name="out", bufs=1) as op,
        ):
            w_sb = constp.tile([PART, KT * MAX_GS], bf16, tag="w")
            nc.sync.dma_start(w_sb[:], wT[:])
            b_sb = constp.tile([PART, 1], f32, tag="b")
            nc.sync.dma_start(b_sb[:], bias2[:])

            # One [128, ns] psum per PAIR of n-chunks: even chunk at
            # partitions 0:64, odd chunk at 64:128 (PE row-offset write).
            psums = []
            for i in range(0, len(offs), 2):
                ns = offs[i][1]
                psums.append(pp.tile([PART, ns], f32, tag=f"ps{i}", name=f"ps{i}"))

            def ps_slice(n):
                pair, half = divmod(n, 2)
                lo = half * MAX_GS
                return psums[pair][lo : lo + MAX_GS, : offs[n][1]]

            # The LDWEIGHTS ISA slot encodes at most one semaphore wait, so
            # no matmul may depend on two DMAs at once. Absorb the w/b DMA
            # waits into throwaway ops so each real matmul waits only on its
            # h-chunk DMA (and the first tensor_scalar_add only on PE).
            ps_warm = pp.tile([MAX_GS, MAX_GS], f32, tag="pswarm", name="pswarm")
            nc.tensor.matmul(
                ps_warm[:, :], w_sb[:, 0:MAX_GS], w_sb[:, 0:MAX_GS],
                start=True, stop=True,
            )
            b_warm = constp.tile([PART, 1], f32, tag="bwarm", name="bwarm")
            nc.vector.tensor_copy(b_warm[:], b_sb[:])

            for ic in range(KT // KCH):
                h_sb = hp.tile([PART, KCH * C], bf16, tag="h")
                for tl in range(KCH):
                    ring(ic).dma_start(
                        h_sb[:, tl * C : (tl + 1) * C], hT[ic * KCH + tl]
                    )
                for tl in range(KCH):
                    t = ic * KCH + tl
                    for n, (no, ns) in enumerate(offs):
                        nc.tensor.matmul(
                            ps_slice(n),
                            w_sb[:, t * MAX_GS : (t + 1) * MAX_GS],
                            h_sb[:, tl * C + no : tl * C + no + ns],
                            start=(t == 0),
                            stop=(t == KT - 1),
                        )

            o_sb = op.tile([PART, C2], f32, tag="o")
            if len(offs) % 2 == 1:
                # odd chunk count: partitions 64:128 of the last pair-column
                # block are never written -- zero them so the packed DMA
                # reads initialized SBUF.
                nc.vector.memset(o_sb[MAX_GS:PART, C2 - offs[-1][1] : C2], 0)
            col = 0
            for i in range(0, len(offs), 2):
                ns = offs[i][1]
                pr = PART if i + 1 < len(offs) else MAX_GS
                nc.vector.tensor_scalar_add(
                    o_sb[:pr, col : col + ns], psums[i // 2][:pr, :], b_sb[:pr]
                )
                col += ns
            nc.gpsimd.dma_start(outT[:], o_sb[:])

    return nc


def kernel(**inputs):
    global LAST_EXEC_NS, LAST_RESULTS
    from concourse.bass_utils import run_bass_kernel_spmd

    hidden = np.ascontiguousarray(np.asarray(inputs["hidden"], dtype=np.float32))
    idx = np.asarray(inputs["chosen_group_idx"]).astype(np.int64)
    W = np.asarray(inputs["W"], dtype=np.float32)
    b = np.asarray(inputs["b"], dtype=np.float32)
    gs = np.asarray(inputs["group_sizes"])

    B = hidden.shape[0]
    counts = np.bincount(idx, minlength=N_GROUPS)
    C = max(PART, int(-(-counts.max() // PART)) * PART)
    offs, C2 = _chunks(C)

    positions = [np.nonzero(idx == g)[0] for g in range(N_GROUPS)]

    bf = ml_dtypes.bfloat16
    in_maps = []
    for g in range(N_GROUPS):
        pos = positions[g]
        hg = np.zeros((C, D_MODEL), bf)
        hg[: len(pos)] = hidden[pos, g, :].astype(bf)
        hT = np.ascontiguousarray(hg.T).reshape(KT, PART, C)
        wT = np.ascontiguousarray(
            W[g].reshape(MAX_GS, KT, PART).transpose(2, 1, 0)
        ).reshape(PART, KT * MAX_GS).astype(bf)
        bias2 = np.ascontiguousarray(
            np.concatenate([b[g], b[g]])[:, None]
        )
        in_maps.append({"hT": hT, "wT": wT, "bias2": bias2})

    if C not in _nc_cache:
        _nc_cache[C] = _build_nc(C)
    nc = _nc_cache[C]

    res = run_bass_kernel_spmd(nc, in_maps, list(range(N_GROUPS)), trace=TRACE)
    LAST_EXEC_NS = res.exec_time_ns
    LAST_RESULTS = res

    preds = np.zeros((B, MAX_GS), np.float32)
    for g in range(N_GROUPS):
        pos = positions[g]
        outT = res.results[g]["outT"]  # [128, C2]
        # unpack: pair i occupies cols [sum of even sizes before it],
        # even chunk at partitions 0:64, odd chunk at 64:128
        og = np.zeros((C, MAX_GS), np.float32)
        col = 0
        for i in range(0, len(offs), 2):
            no, ns = offs[i]
            og[no : no + ns] = outT[0:MAX_GS, col : col + ns].T
            if i + 1 < len(offs):
                no1, ns1 = offs[i + 1]
                og[no1 : no1 + ns1] = outT[MAX_GS:PART, col : col + ns1].T
            col += ns
        preds[pos] = og[: len(pos)]

    valid = np.arange(MAX_GS)[None, :] < gs[idx][:, None]
    preds = np.where(valid, preds, np.float32(0.0))
    return preds, valid


# revision 13
# speedup vs baseline: 1.0392x; 1.0392x over previous
"""GroupPretrainHead on 8 NeuronCores (Trainium2, Bass/Tile).

Expert-parallel sharding: core g owns group g's decoder (W[g], b[g]) and
processes exactly the samples routed to group g. The host does the routing
permutation (gather/scatter of rows = the MoE dispatch/combine step); the
device does all FLOPs: out.T = W[g] @ h.T + b[g] as a K-accumulated matmul.

v4 (from v2/v3 trace analysis):
 - h/W/out in bf16 (PSUM accumulation stays fp32), bias f32.
 - hT streamed one k-tile per DMA, strictly alternating the two HWDGE
   rings (sync/scalar) so k-tiles arrive in consumption order; each ring
   sustains ~160 GB/s and together they run near the ~358 GB/s HBM cap.
   Coarser chunks measurably regress: packets of queued DMAs interleave
   per-engine, so a big leading chunk completes near stream end and
   starves the PE (v3: 35us vs v2: 31.8us).
 - w / bias / packed output ride the SWDGE (gpsimd) path: fresh DMA lanes
   with exactly one semaphore wait each (this walrus rejects >1 wait per
   instruction), and they keep all 8 HWDGE lanes for the h stream.
 - output packed [128, C2]: n-chunk pairs stacked two-deep in partitions
   (PE writes the odd chunk at PSUM partition offset 64), single
   full-width bf16 writeback.
 - kernel tail: drop the redundant semaphore clear + second barrier (the
   framework epilogue zeroes every semaphore right after anyway).

Device-side layout per core (C = max group count, rounded up to 32):
  hT    [16, 128, C]  bf16 -- gathered hidden rows, transposed, k-major
  wT    [128, 16*64]  bf16 -- W[g] transposed to [d-partition, (ktile j)]
  bias2 [128, 1]      f32  -- b[g] stacked twice
  outT  [128, C2]     bf16 -- preds.T, chunk pairs at partitions 0/64
"""

import numpy as np
import ml_dtypes

N_GROUPS = 8
D_MODEL = 2048
MAX_GS = 64
PART = 128
KT = D_MODEL // PART  # 16
NCH = 512  # matmul n-chunk (one PSUM bank of f32)

TRACE = False
LAST_EXEC_NS = None
LAST_RESULTS = None

_nc_cache = {}


def _chunks(C):
    """n-chunk (offset, size) pairs and the packed outT column count."""
    offs = [(o, min(NCH, C - o)) for o in range(0, C, NCH)]
    c2 = 0
    for i in range(0, len(offs), 2):
        c2 += offs[i][1]
    return offs, c2


def _make_tile_context_cls():
    import concourse.mybir as mybir
    from concourse.tile import TileContext
    from concourse.vector_clock import ScopedClock

    class SplitDrainTileContext(TileContext):
        """This container's walrus encodes at most ONE semaphore wait per
        instruction; Tile's kernel-tail drain aggregates every outstanding
        sem onto a single InstDrain, which fails codegen. Split it into a
        chain of one-wait drains. Also skip the per-kernel semaphore clear
        and the second barrier: the framework epilogue zeroes all 256
        semaphores right after this block in every NEFF iteration."""

        def _drain_and_barrier(self, tick_clock, wait_clock):
            drain_inst = self.nc.sync.drain()
            wait_clock.add_sem_waits(
                drain_inst.ins, ScopedClock({None: tick_clock.global_clock})
            )
            si = drain_inst.ins.sync_info
            waits = list(si.on_wait) if si else []
            if len(waits) > 1:
                si.on_wait = waits[:1]
                drain_inst.ins.sync_info = si
                for w in waits[1:]:
                    d2 = self.nc.sync.drain()
                    d2.ins.sync_info = mybir.SyncInfo(on_wait=[w], on_update=[])
            self.nc.all_engine_barrier()
            popped = self.nc._tile_sem_poison_stack.pop()
            assert popped is self._sem_poison

    return SplitDrainTileContext


def _build_nc(C):
    import concourse.bass as bass
    import concourse.mybir as mybir

    TileContext = _make_tile_context_cls()

    f32 = mybir.dt.float32
    bf16 = mybir.dt.bfloat16
    nc = bass.Bass()

    offs, C2 = _chunks(C)
    KTM = KT * MAX_GS

    hT = nc.declare_dram_parameter("hT", [KT, PART, C], bf16, isOutput=False)
    wT = nc.declare_dram_parameter("wT", [PART, KTM], bf16, isOutput=False)
    bias2 = nc.declare_dram_parameter("bias2", [PART, 1], f32, isOutput=False)
    outT = nc.declare_dram_parameter("outT", [PART, C2], bf16, isOutput=True)

    # The two HWDGE rings: SP (sync) and ACT (scalar).
    def ring(i):
        return nc.sync if i % 2 == 0 else nc.scalar

    with TileContext(nc) as tc:
        with (
            tc.tile_pool(name="const", bufs=1) as constp,
            tc.tile_pool(name="h", bufs=KT) as hp,
            tc.tile_pool(name="psum", bufs=1, space=bass.MemorySpace.PSUM) as pp,
            tc.tile_pool(name="out", bufs=1) as op,
        ):
            # w and bias go over SWDGE so the HWDGE lanes are h-only.
            w_sb = constp.tile([PART, KTM], bf16, tag="w")
            nc.gpsimd.dma_start(w_sb[:], wT[:])
            b_sb = constp.tile([PART, 1], f32, tag="bias")
            nc.gpsimd.dma_start(b_sb[:], bias2[:])

            # One [128, ns] psum per PAIR of n-chunks: even chunk at
            # partitions 0:64, odd chunk at 64:128 (PE row-offset write).
            psums = []
            for i in range(0, len(offs), 2):
                pr = PART if i + 1 < len(offs) else MAX_GS
                psums.append(
                    pp.tile([pr, offs[i][1]], f32, tag=f"ps{i}", name=f"ps{i}")
                )

            def ps_slice(n):
                pair, half = divmod(n, 2)
                lo = half * MAX_GS
                return psums[pair][lo : lo + MAX_GS, : offs[n][1]]

            # Absorb the w/bias DMA waits into throwaway ops on PE and DVE
            # so each real matmul waits only on its h-tile DMA and the first
            # tensor_scalar_add waits only on PE (one-wait-per-inst limit).
            ps_warm = pp.tile([MAX_GS, MAX_GS], f32, tag="pswarm", name="pswarm")
            nc.tensor.matmul(
                ps_warm[:, :], w_sb[:, 0:MAX_GS], w_sb[:, 0:MAX_GS],
                start=True, stop=True,
            )
            b_warm = constp.tile([PART, 1], f32, tag="bwarm", name="bwarm")
            nc.vector.tensor_copy(b_warm[:], b_sb[:])

            for t in range(KT):
                h_sb = hp.tile([PART, C], bf16, tag="h")
                ring(t).dma_start(h_sb[:], hT[t])
                for n in range(len(offs)):
                    no, ns = offs[n]
                    nc.tensor.matmul(
                        ps_slice(n),
                        w_sb[:, t * MAX_GS : (t + 1) * MAX_GS],
                        h_sb[:, no : no + ns],
                        start=(t == 0),
                        stop=(t == KT - 1),
                    )

            o_sb = op.tile([PART, C2], bf16, tag="o")
            if len(offs) % 2 == 1:
                # odd chunk count: partitions 64:128 of the last column
                # block are never written -- zero them so the packed DMA
                # reads initialized SBUF.
                nc.vector.memset(o_sb[MAX_GS:PART, C2 - offs[-1][1] : C2], 0)
            col = 0
            for i in range(0, len(offs), 2):
                ns = offs[i][1]
                pr = PART if i + 1 < len(offs) else MAX_GS
                nc.vector.tensor_scalar_add(
                    o_sb[:pr, col : col + ns], psums[i // 2][:pr, :], b_sb[:pr]
                )
                col += ns
            nc.gpsimd.dma_start(outT[:], o_sb[:])

    return nc


def kernel(**inputs):
    global LAST_EXEC_NS, LAST_RESULTS
    from concourse.bass_utils import run_bass_kernel_spmd

    hidden = np.ascontiguousarray(np.asarray(inputs["hidden"], dtype=np.float32))
    idx = np.asarray(inputs["chosen_group_idx"]).astype(np.int64)
    W = np.asarray(inputs["W"], dtype=np.float32)
    b = np.asarray(inputs["b"], dtype=np.float32)
    gs = np.asarray(inputs["group_sizes"])

    B = hidden.shape[0]
    counts = np.bincount(idx, minlength=N_GROUPS)
    C = max(1056, int(-(-counts.max() // 32)) * 32)
    offs, C2 = _chunks(C)

    positions = [np.nonzero(idx == g)[0] for g in range(N_GROUPS)]

    bf = ml_dtypes.bfloat16
    in_maps = []
    for g in range(N_GROUPS):
        pos = positions[g]
        hg = np.zeros((C, D_MODEL), bf)
        hg[: len(pos)] = hidden[pos, g, :].astype(bf)
        hT = np.ascontiguousarray(hg.T).reshape(KT, PART, C)
        wT = np.ascontiguousarray(
            W[g].reshape(MAX_GS, KT, PART).transpose(2, 1, 0)
        ).reshape(PART, KT * MAX_GS).astype(bf)
        bias2 = np.ascontiguousarray(
            np.concatenate([b[g], b[g]])[:, None].astype(np.float32)
        )
        in_maps.append({"hT": hT, "wT": wT, "bias2": bias2})

    if C not in _nc_cache:
        _nc_cache[C] = _build_nc(C)
    nc = _nc_cache[C]

    res = run_bass_kernel_spmd(nc, in_maps, list(range(N_GROUPS)), trace=TRACE)
    LAST_EXEC_NS = res.exec_time_ns
    LAST_RESULTS = res

    preds = np.zeros((B, MAX_GS), np.float32)
    for g in range(N_GROUPS):
        pos = positions[g]
        outT = np.asarray(res.results[g]["outT"]).astype(np.float32)  # [128, C2]
        og = np.zeros((C, MAX_GS), np.float32)
        col = 0
        for i in range(0, len(offs), 2):
            no, ns = offs[i]
            og[no : no + ns] = outT[0:MAX_GS, col : col + ns].T
            if i + 1 < len(offs):
                no1, ns1 = offs[i + 1]
                og[no1 : no1 + ns1] = outT[MAX_GS:PART, col : col + ns1].T
            col += ns
        preds[pos] = og[: len(pos)]

    valid = np.arange(MAX_GS)[None, :] < gs[idx][:, None]
    preds = np.where(valid, preds, np.float32(0.0))
    return preds, valid
